# revision 20
# baseline (speedup 1.0000x reference)
"""GAU (gated attention unit) Trainium2 kernel, 8-way SPMD over the sequence dim.

Problem (fp32):
    h    = LayerNorm(x) * gamma + beta            x: [4096, 1024]
    uv   = silu(h @ uv_w.T + uv_b)                uv: [4096, 4224] = [u | v | base]
    q, k = base * qk_w[0,1] + qk_b[0,1]           base: [4096, 128]
    g    = relu(q @ k.T / sqrt(128))^2            g: [4096, 4096]
    out  = (u * (g @ v)) @ o_w.T + o_b + x        out: [4096, 1024]

Sharding: rows (sequence) split 8 ways; each core computes its own 512-row
slice of everything; k and v are AllGathered across the 8 cores in 5 small
pipelined collectives (k first, then 4 chunks of 512 v columns) so the
transfers hide behind the u/scores compute. A zero-byte dummy AllGather is
fired first thing to absorb the runtime's first-collective rendezvous
barrier while the LayerNorm still runs.

All large matmuls run fp8(e4m3) in DoubleRow perf mode (256-row
contraction, 2x PE throughput): the uv projection, the attention g @ v,
and the output projection. Scores run fp8 non-DR (S=128 contraction).
Scale management (all folded on the host / into activation scales):
  uv/o weights lifted x64 into fp8 range (silu input scale 2^-6 undoes it),
  q,k carry x16 each -> scores x2^8, g = relu(qk)^2 x2^16, o_w lift x2^6,
  final copy-scale 2^-22 restores true magnitude before the residual add.
The residual path (x + o_b, precomputed on host) stays fp32, so the fp8
rounding only touches the attention contribution, which is orders of
magnitude below the fp32 residual.
"""
import sys

sys.path.insert(0, "/opt/trn_rl_repo")

import numpy as np
import concourse.bass as bass
import concourse.tile as tile
from concourse import bacc, mybir
from concourse.bass_utils import run_bass_kernel_spmd

F32 = mybir.dt.float32
BF16 = mybir.dt.bfloat16
F8 = mybir.dt.float8e4
DR = mybir.MatmulPerfMode.DoubleRow
AF = mybir.ActivationFunctionType
OP = mybir.AluOpType

N_CORES = 8
N = 4096          # sequence
H = 1024          # hidden
E = 2048          # expansion
S = 128           # qk dim
UV = 2 * E + S    # 4224
R = N // N_CORES  # 512 rows per core
P = 128
EPS = 1e-5

HT = H // P       # 8  h-tiles
HP = HT // 2      # 4  h-tile pairs (DR contraction)
RT = R // P       # 4  row tiles per core
UT = E // P       # 16 u col tiles
KT = N // P       # 32 key tiles
VCH = 4           # v column chunks
VC = E // VCH     # 512 cols per chunk

WLIFT = 64.0            # fp8 weight lift (uv_w, o_w)
ISCALE = 1.0 / WLIFT    # activation input scale undoing the lift
QKS = 16.0              # per-operand q/k scale
OSCALE = 2.0 ** -22     # (QKS^2)^2 * WLIFT undone at the output


def _bcast_load(nc, sbuf_tile, dram_ap):
    """DMA a DRAM vector to all partitions (partition stride 0)."""
    nc.scalar.dma_start(out=sbuf_tile,
                        in_=dram_ap.partition_broadcast(sbuf_tile.shape[0]))


def build():
    nc = bacc.Bacc("TRN2", target_bir_lowering=False, debug=False,
                   num_devices=N_CORES)

    # ---- kernel I/O (per core) ----
    xt = nc.declare_dram_parameter("xt", [H, R], F32, isOutput=False)
    xpb_d = nc.declare_dram_parameter("xpb", [R, H], F32, isOutput=False)
    uv_wt = nc.declare_dram_parameter("uv_wt", [H, UV], F8, isOutput=False)
    o_wt = nc.declare_dram_parameter("o_wt", [E, H], F8, isOutput=False)
    uvb_d = nc.declare_dram_parameter("uvb", [UV], F32, isOutput=False)
    uvbv_d = nc.declare_dram_parameter("uvb_v64", [E], F32, isOutput=False)
    qs_d = nc.declare_dram_parameter("qs", [S], F32, isOutput=False)
    qb_d = nc.declare_dram_parameter("qb", [S], F32, isOutput=False)
    ks_d = nc.declare_dram_parameter("ks", [S], F32, isOutput=False)
    kb_d = nc.declare_dram_parameter("kb", [S], F32, isOutput=False)
    out = nc.declare_dram_parameter("out", [R, H], F32, isOutput=True)

    xtr = xt.ap()
    uv_wtr = uv_wt.ap()
    o_wtr = o_wt.ap()
    outr = out.ap()

    from contextlib import ExitStack
    with tile.TileContext(nc) as tc, ExitStack() as ctx:
        singles = ctx.enter_context(tc.tile_pool(name="singles", bufs=1))
        wpool = ctx.enter_context(tc.tile_pool(name="wpool", bufs=2))
        tmp = ctx.enter_context(tc.tile_pool(name="tmp", bufs=2))
        ps = ctx.enter_context(tc.tile_pool(name="ps", bufs=8, space="PSUM"))
        dram = ctx.enter_context(tc.tile_pool(name="dram", bufs=1,
                                              space="DRAM"))

        # ---- internal DRAM for collectives ----
        # v contribution j is packed [128, 4*VC]: row p carries the chunk's
        # VC columns for local rows p, 128+p, 256+p, 384+p side by side, so
        # both the store and the post-gather reload move 2KB-contiguous
        # rows (512B rows measured ~3x slower).
        # k goes first as its own tiny gather: the scores (and through them
        # the whole back half) gate on it, while the v chunks are only
        # needed once stage 4 spins up.
        k_c = dram.tile([P, R], F8)
        k_g = dram.tile([N_CORES * P, R], F8, addr_space="Shared")
        v_c = [dram.tile([P, RT * VC], F8, name=f"v_c{j}")
               for j in range(VCH)]
        v_g = [dram.tile([N_CORES * P, RT * VC], F8, addr_space="Shared",
                         name=f"v_g{j}")
               for j in range(VCH)]

        # ---- constants / biases (small loads on the scalar queue) ----
        eps_t = singles.tile([P, 1], F32)
        nc.vector.memset(eps_t, EPS)
        uvb_u = singles.tile([P, UT + 1], F32)
        nc.scalar.dma_start(uvb_u[:, :UT],
                            uvb_d.ap()[:E].rearrange("(t p) -> p t", p=P))
        nc.scalar.dma_start(uvb_u[:, UT:UT + 1],
                            uvb_d.ap()[2 * E:].rearrange("(t p) -> p t", p=P))
        qs_t = singles.tile([P, 1], F32)
        nc.scalar.dma_start(qs_t, qs_d.ap().rearrange("(t p) -> p t", p=P))
        qb_t = singles.tile([P, 1], F32)
        nc.scalar.dma_start(qb_t, qb_d.ap().rearrange("(t p) -> p t", p=P))
        ks_t = singles.tile([P, 1], F32)
        nc.scalar.dma_start(ks_t, ks_d.ap().rearrange("(t p) -> p t", p=P))
        kb_t = singles.tile([P, 1], F32)
        nc.scalar.dma_start(kb_t, kb_d.ap().rearrange("(t p) -> p t", p=P))
        # v bias broadcast (x64 psum domain), on the scalar queue
        uvb_v_bc = singles.tile([P, E], F32)
        _bcast_load(nc, uvb_v_bc, uvbv_d.ap())

        # ---- persistent activations ----
        hT = singles.tile([P, HT, R], F8)     # transposed LN output (fp8)
        uT = singles.tile([P, UT, R], F8)     # u, later y = u*attn in place
        baseT = singles.tile([P, R], F32)
        qT = singles.tile([P, R], F8)
        kT_sb = singles.tile([P, R], F8)
        kT_full = singles.tile([P, KT // RT, R], F8)   # [S, chunk, key]
        g_sb = singles.tile([P, KT, R], F8)            # [key, kt, row]
        wo = singles.tile([P, UT, H], F8)              # o weights, whole

        # ================= Stage 1: LayerNorm (transposed layout) =========
        # x arrives host-transposed as xT [H, R]; stats are computed by
        # contracting the partition (hidden) dim with a ones vector on the
        # PE (fp32 sum chain — the tensor engine is idle this early), then
        # broadcast back over partitions. The normalize is split: subtract
        # on gpsimd (can start as soon as mu lands), multiply on vector.
        ones_f = singles.tile([P, P], F32)
        nc.vector.memset(ones_f, 1.0)
        ones_b = singles.tile([P, P], BF16)
        nc.vector.memset(ones_b, 1.0)
        xT = singles.tile([P, HT, R], F32)
        xsq = singles.tile([P, HT, R], BF16)
        xtr3 = xtr[:].rearrange("(t p) r -> p t r", p=P)
        for hc in range(4):
            nc.sync.dma_start(xT[:, 2 * hc:2 * hc + 2, :],
                              xtr3[:, 2 * hc:2 * hc + 2, :])
        for ht in range(HT):
            nc.vector.tensor_tensor(xsq[:, ht, :], xT[:, ht, :],
                                    xT[:, ht, :], OP.mult)
        psum_s = ps.tile([P, R], F32, tag="mm", name="psum_s")
        psum_q = ps.tile([P, R], F32, tag="mm", name="psum_q")
        for ht in range(HT):
            nc.tensor.matmul(psum_s, ones_f, xT[:, ht, :],
                             start=(ht == 0), stop=(ht == HT - 1))
        for ht in range(HT):
            nc.tensor.matmul(psum_q, ones_b, xsq[:, ht, :],
                             start=(ht == 0), stop=(ht == HT - 1))
        mu_bc = singles.tile([P, R], F32)
        nc.vector.tensor_scalar_mul(mu_bc, psum_s, 1.0 / H)
        rstd_bc = singles.tile([P, R], F32)
        nc.vector.tensor_scalar_mul(rstd_bc, psum_q, 1.0 / H)
        mu2 = singles.tile([P, R], F32)
        nc.vector.tensor_tensor(mu2, mu_bc, mu_bc, OP.mult)
        nc.vector.tensor_tensor(rstd_bc, rstd_bc, mu2, OP.subtract)
        nc.scalar.activation(out=rstd_bc, in_=rstd_bc, func=AF.Sqrt,
                             bias=eps_t, scale=1.0)
        nc.vector.reciprocal(out=rstd_bc, in_=rstd_bc)
        for ht in range(HT):
            nc.vector.tensor_tensor(xT[:, ht, :], xT[:, ht, :], mu_bc,
                                    OP.subtract)
            nc.vector.tensor_tensor(hT[:, ht, :], xT[:, ht, :], rstd_bc,
                                    OP.mult)

        def proj_mm(psum, w_pairs_of, moving_rows=None):
            """4 DR matmuls accumulating h-pair contractions into psum."""
            for hp in range(HP):
                mov = (hT[:, 2 * hp:2 * hp + 2, :] if moving_rows is None
                       else hT[:, 2 * hp:2 * hp + 2, moving_rows])
                nc.tensor.matmul(psum, w_pairs_of(hp), mov, perf_mode=DR,
                                 start=(hp == 0), stop=(hp == HP - 1))

        # ================= Stage 2a: base -> q,k; fire k gather ===========
        wbase = singles.tile([P, HT, S], F8)
        nc.sync.dma_start(wbase,
                          uv_wtr[:, 2 * E:].rearrange("(t p) c -> p t c", p=P))
        pb = ps.tile([P, R], F32, tag="mm")
        proj_mm(pb, lambda hp: wbase[:, 2 * hp:2 * hp + 2, :])
        nc.scalar.activation(out=baseT, in_=pb, func=AF.Silu,
                             bias=uvb_u[:, UT:UT + 1], scale=ISCALE)
        nc.vector.tensor_scalar(out=qT, in0=baseT, scalar1=qs_t, scalar2=qb_t,
                                op0=OP.mult, op1=OP.add)
        nc.vector.tensor_scalar(out=kT_sb, in0=baseT, scalar1=ks_t,
                                scalar2=kb_t, op0=OP.mult, op1=OP.add)
        nc.gpsimd.dma_start(k_c[:], kT_sb)
        nc.gpsimd.collective_compute(
            "AllGather", OP.bypass, replica_groups=[list(range(N_CORES))],
            ins=[k_c.opt()], outs=[k_g.opt()])

        # ================= Stage 2b: v (natural layout), chunked gathers ==
        # v psum is [rows, vcols] (hT pairs stationary, weights moving);
        # each 512-col chunk is stored+gathered as soon as it's done.
        # gpsimd carries ONLY contribution stores + triggers; the
        # gather-dependent reloads ride the idle sync/scalar HWDGE queues.
        for j in range(VCH):
            wv = wpool.tile([P, HT, VC], F8, tag="wuv")
            nc.sync.dma_start(
                wv, uv_wtr[:, E + j * VC:E + (j + 1) * VC]
                .rearrange("(t p) c -> p t c", p=P))
            v_sb = wpool.tile([P, RT, VC], F8, tag="vsb", name=f"v_sb{j}")
            for rt in range(RT):
                pv = ps.tile([P, VC], F32, tag="mm")
                for hp in range(HP):
                    nc.tensor.matmul(
                        pv, hT[:, 2 * hp:2 * hp + 2, rt * P:(rt + 1) * P],
                        wv[:, 2 * hp:2 * hp + 2, :], perf_mode=DR,
                        start=(hp == 0), stop=(hp == HP - 1))
                vtmp = tmp.tile([P, VC], F32, tag="vtmp")
                nc.vector.tensor_tensor(vtmp, pv,
                                        uvb_v_bc[:, j * VC:(j + 1) * VC],
                                        OP.add)
                nc.scalar.activation(out=v_sb[:, rt, :], in_=vtmp,
                                     func=AF.Silu, scale=ISCALE)
            nc.gpsimd.dma_start(
                v_c[j][:].rearrange("p (t c) -> p t c", t=RT), v_sb)
            nc.gpsimd.collective_compute(
                "AllGather", OP.bypass,
                replica_groups=[list(range(N_CORES))],
                ins=[v_c[j].opt()], outs=[v_g[j].opt()])

        # ================= Stage 2c: u (fills the gather shadow) ==========
        for ug in range(4):
            wu = wpool.tile([P, HT, 512], F8, tag="wuv")
            nc.sync.dma_start(
                wu, uv_wtr[:, ug * 512:(ug + 1) * 512]
                .rearrange("(t p) c -> p t c", p=P))
            for ui in range(4):
                ut = ug * 4 + ui
                pu = ps.tile([P, R], F32, tag="mm")
                proj_mm(pu, lambda hp: wu[:, 2 * hp:2 * hp + 2,
                                          ui * P:(ui + 1) * P])
                nc.scalar.activation(out=uT[:, ut, :], in_=pu, func=AF.Silu,
                                     bias=uvb_u[:, ut:ut + 1], scale=ISCALE)

        # o weights + residual: loaded behind the u weights on sync
        nc.sync.dma_start(wo, o_wtr[:].rearrange("(t p) c -> p t c", p=P))
        xpb = singles.tile([P, RT, H], F32)
        nc.sync.dma_start(xpb, xpb_d.ap().rearrange("(t p) c -> p t c", p=P))

        # gather reloads: vchunks on sync (idle until the out stores),
        # kT_full on scalar (only gates the score relus, which need the
        # gather anyway). 2KB-contiguous rows on both sides.
        vchunks = []
        for j in range(VCH):
            vchunk = wpool.tile([P, N_CORES, RT, VC], F8, tag="vchunk",
                                name=f"vchunk{j}")
            nc.sync.dma_start(
                vchunk[:].rearrange("p c q e -> p c (q e)"),
                v_g[j][:].rearrange("(c p) r -> p c r", p=P))
            vchunks.append(vchunk)
        nc.scalar.dma_start(
            kT_full, k_g[:].rearrange("(c p) r -> p c r", p=P))

        # ================= Stage 3: scores + relu^2 =======================
        # kT_full rows for core c live at k_g[c*P:(c+1)*P].
        for kt in range(KT):
            c, rb = kt // RT, kt % RT
            pg = ps.tile([P, R], F32, tag="mm")
            nc.tensor.matmul(pg, kT_full[:, c, rb * P:(rb + 1) * P],
                             qT[:], start=True, stop=True)
            t_relu = tmp.tile([P, R], F32, tag="relu", bufs=4)
            nc.scalar.activation(out=t_relu, in_=pg, func=AF.Relu)
            nc.vector.tensor_tensor(g_sb[:, kt, :], t_relu, pg, OP.mult)

        # ================= Stage 4: attn = g @ v; y = u * attn ===========
        # fp8 DoubleRow: stationary = v key-pair stripes, moving = g pairs.
        for j in range(VCH):
            vchunk = vchunks[j]
            pa = [ps.tile([P, R], F32, tag="mm", name=f"pa{j}_{ei}")
                  for ei in range(VC // P)]
            for kp in range(KT // 2):
                c8, rp = kp // 2, kp % 2
                gpair = g_sb[:, 2 * kp:2 * kp + 2, :]
                for ei in range(VC // P):
                    nc.tensor.matmul(
                        pa[ei], vchunk[:, c8, 2 * rp:2 * rp + 2,
                                       ei * P:(ei + 1) * P],
                        gpair, perf_mode=DR,
                        start=(kp == 0), stop=(kp == KT // 2 - 1))
            for ei in range(VC // P):
                et = j * (VC // P) + ei
                nc.vector.tensor_tensor(uT[:, et, :], pa[ei], uT[:, et, :],
                                        OP.mult)

        # ================= Stage 5: out = y @ o_w.T * 2^-22 + (x + o_b) ==
        for hc in range(2):
            for rt in range(RT):
                po = ps.tile([P, 512], F32, tag="mm")
                for ep in range(UT // 2):
                    nc.tensor.matmul(
                        po, uT[:, 2 * ep:2 * ep + 2, rt * P:(rt + 1) * P],
                        wo[:, 2 * ep:2 * ep + 2, hc * 512:(hc + 1) * 512],
                        perf_mode=DR,
                        start=(ep == 0), stop=(ep == UT // 2 - 1))
                o_sb = tmp.tile([P, 512], F32, tag="osb")
                nc.scalar.mul(o_sb, po, OSCALE)
                nc.vector.tensor_tensor(o_sb, o_sb,
                                        xpb[:, rt, hc * 512:(hc + 1) * 512],
                                        OP.add)
                nc.sync.dma_start(
                    outr[rt * P:(rt + 1) * P, hc * 512:(hc + 1) * 512], o_sb)

    nc.finalize()
    return nc


_NC_CACHE = None


def _get_nc():
    global _NC_CACHE
    if _NC_CACHE is None:
        _NC_CACHE = build()
    return _NC_CACHE


def _f8(a):
    import ml_dtypes
    return np.ascontiguousarray(
        np.clip(a, -240.0, 240.0)).astype(ml_dtypes.float8_e4m3fn)


def _make_in_maps(inputs):
    x = np.ascontiguousarray(inputs["x"], dtype=np.float32)
    uv_w = np.asarray(inputs["uv_w"], dtype=np.float32)
    o_w = np.asarray(inputs["o_w"], dtype=np.float32)
    qk_w = np.asarray(inputs["qk_weight"], dtype=np.float32)
    qk_b = np.asarray(inputs["qk_bias"], dtype=np.float32)
    gamma = np.asarray(inputs["ln_gamma"], dtype=np.float32)
    beta = np.asarray(inputs["ln_beta"], dtype=np.float32)
    uv_b = np.asarray(inputs["uv_b"], dtype=np.float32)
    o_b = np.asarray(inputs["o_b"], dtype=np.float32)
    scale = np.float32(1.0 / np.sqrt(np.float32(128.0)))

    # fold gamma/beta into the uv projection:
    #   (z*gamma + beta) @ W.T = z @ (W*gamma).T + W@beta
    uv_w_f = uv_w * gamma[None, :]
    uv_b_f = (uv_b.astype(np.float64)
              + uv_w.astype(np.float64) @ beta.astype(np.float64)
              ).astype(np.float32)

    shared = dict(
        uv_wt=_f8(uv_w_f.T * WLIFT),
        o_wt=_f8(o_w.T * WLIFT),
        uvb=np.ascontiguousarray(uv_b_f),
        uvb_v64=np.ascontiguousarray(uv_b_f[E:2 * E] * WLIFT),
        qs=np.ascontiguousarray(qk_w[0] * scale * QKS),
        qb=np.ascontiguousarray(qk_b[0] * scale * QKS),
        ks=np.ascontiguousarray(qk_w[1] * QKS),
        kb=np.ascontiguousarray(qk_b[1] * QKS),
    )
    return [dict(shared,
                 xt=np.ascontiguousarray(x[c * R:(c + 1) * R].T),
                 xpb=np.ascontiguousarray(x[c * R:(c + 1) * R] + o_b))
            for c in range(N_CORES)]


def run(inputs, trace=False, **kw):
    nc = _get_nc()
    in_maps = _make_in_maps(inputs)
    res = run_bass_kernel_spmd(nc, in_maps, list(range(N_CORES)),
                               trace=trace, **kw)
    out = np.concatenate([res.results[c]["out"] for c in range(N_CORES)],
                         axis=0)
    return out, res


def kernel(**inputs) -> np.ndarray:
    out, _ = run(inputs)
    return out


# revision 25
# speedup vs baseline: 1.2904x; 1.2904x over previous
"""GAU (gated attention unit) Trainium2 kernel, 8-way SPMD over the sequence dim.

Problem (fp32):
    h    = LayerNorm(x) * gamma + beta            x: [4096, 1024]
    uv   = silu(h @ uv_w.T + uv_b)                uv: [4096, 4224] = [u | v | base]
    q, k = base * qk_w[0,1] + qk_b[0,1]           base: [4096, 128]
    g    = relu(q @ k.T / sqrt(128))^2            g: [4096, 4096]
    out  = (u * (g @ v)) @ o_w.T + o_b + x        out: [4096, 1024]

Sharding: rows (sequence) split 8 ways; each core computes its own 512-row
slice of everything; k and v are AllGathered across the 8 cores in 5 small
pipelined collectives (k first, then 4 chunks of 512 v columns) so the
transfers hide behind the u/scores compute. A zero-byte dummy AllGather is
fired first thing to absorb the runtime's first-collective rendezvous
barrier while the LayerNorm still runs.

All large matmuls run fp8(e4m3) in DoubleRow perf mode (256-row
contraction, 2x PE throughput): the uv projection, the attention g @ v,
and the output projection. Scores run fp8 non-DR (S=128 contraction).
Scale management (all folded on the host / into activation scales):
  uv/o weights lifted x64 into fp8 range (silu input scale 2^-6 undoes it),
  q,k carry x16 each -> scores x2^8, g = relu(qk)^2 x2^16, o_w lift x2^6,
  final copy-scale 2^-22 restores true magnitude before the residual add.
The residual path (x + o_b, precomputed on host) stays fp32, so the fp8
rounding only touches the attention contribution, which is orders of
magnitude below the fp32 residual.
"""
import sys

sys.path.insert(0, "/opt/trn_rl_repo")

import numpy as np
import concourse.bass as bass
import concourse.tile as tile
from concourse import bacc, mybir
from concourse.bass_utils import run_bass_kernel_spmd

F32 = mybir.dt.float32
BF16 = mybir.dt.bfloat16
F8 = mybir.dt.float8e4
DR = mybir.MatmulPerfMode.DoubleRow
AF = mybir.ActivationFunctionType
OP = mybir.AluOpType

N_CORES = 8
N = 4096          # sequence
H = 1024          # hidden
E = 2048          # expansion
S = 128           # qk dim
UV = 2 * E + S    # 4224
R = N // N_CORES  # 512 rows per core
P = 128
EPS = 1e-5

HT = H // P       # 8  h-tiles
HP = HT // 2      # 4  h-tile pairs (DR contraction)
RT = R // P       # 4  row tiles per core
UT = E // P       # 16 u col tiles
KT = N // P       # 32 key tiles
VCH = 4           # v column chunks
VC = E // VCH     # 512 cols per chunk

WLIFT = 64.0            # fp8 weight lift (uv_w, o_w)
ISCALE = 1.0 / WLIFT    # activation input scale undoing the lift
QKS = 16.0              # per-operand q/k scale
OSCALE = 2.0 ** -22     # (QKS^2)^2 * WLIFT undone at the output


def _bcast_load(nc, sbuf_tile, dram_ap):
    """DMA a DRAM vector to all partitions (partition stride 0)."""
    nc.scalar.dma_start(out=sbuf_tile,
                        in_=dram_ap.partition_broadcast(sbuf_tile.shape[0]))


def build():
    nc = bacc.Bacc("TRN2", target_bir_lowering=False, debug=False,
                   num_devices=N_CORES)

    # ---- kernel I/O (per core) ----
    xt = nc.declare_dram_parameter("xt", [H, R], F32, isOutput=False)
    xpb_d = nc.declare_dram_parameter("xpb", [R, H], F32, isOutput=False)
    uv_wt = nc.declare_dram_parameter("uv_wt", [H, UV], F8, isOutput=False)
    o_wt = nc.declare_dram_parameter("o_wt", [E, H], F8, isOutput=False)
    uvb_d = nc.declare_dram_parameter("uvb", [UV], F32, isOutput=False)
    uvbv_d = nc.declare_dram_parameter("uvb_v64", [E], F32, isOutput=False)
    qs_d = nc.declare_dram_parameter("qs", [S], F32, isOutput=False)
    qb_d = nc.declare_dram_parameter("qb", [S], F32, isOutput=False)
    ks_d = nc.declare_dram_parameter("ks", [S], F32, isOutput=False)
    kb_d = nc.declare_dram_parameter("kb", [S], F32, isOutput=False)
    out = nc.declare_dram_parameter("out", [R, H], F32, isOutput=True)

    xtr = xt.ap()
    uv_wtr = uv_wt.ap()
    o_wtr = o_wt.ap()
    outr = out.ap()

    from contextlib import ExitStack
    with tile.TileContext(nc) as tc, ExitStack() as ctx:
        singles = ctx.enter_context(tc.tile_pool(name="singles", bufs=1))
        wpool = ctx.enter_context(tc.tile_pool(name="wpool", bufs=2))
        tmp = ctx.enter_context(tc.tile_pool(name="tmp", bufs=2))
        ps = ctx.enter_context(tc.tile_pool(name="ps", bufs=8, space="PSUM"))
        dram = ctx.enter_context(tc.tile_pool(name="dram", bufs=1,
                                              space="DRAM"))

        # ---- internal DRAM for collectives ----
        # v contribution j is packed [128, 4*VC]: row p carries the chunk's
        # VC columns for local rows p, 128+p, 256+p, 384+p side by side, so
        # both the store and the post-gather reload move 2KB-contiguous
        # rows (512B rows measured ~3x slower).
        # chunk 0 also carries k (first R bytes of each row): the CC ops
        # cost ~10-20us of fixed latency each on the single serialized CC
        # stream, so riding k in op 1 beats a separate k op.
        kv_c = dram.tile([P, R + RT * VC], F8)
        kv_g = dram.tile([N_CORES * P, R + RT * VC], F8, addr_space="Shared")
        v_c = [None] + [dram.tile([P, RT * VC], F8, name=f"v_c{j}")
                        for j in range(1, VCH)]
        v_g = [None] + [dram.tile([N_CORES * P, RT * VC], F8,
                                  addr_space="Shared", name=f"v_g{j}")
                        for j in range(1, VCH)]

        # ---- constants / biases (small loads on the scalar queue) ----
        eps_t = singles.tile([P, 1], F32)
        nc.vector.memset(eps_t, EPS)
        uvb_u = singles.tile([P, UT + 1], F32)
        nc.scalar.dma_start(uvb_u[:, :UT],
                            uvb_d.ap()[:E].rearrange("(t p) -> p t", p=P))
        nc.scalar.dma_start(uvb_u[:, UT:UT + 1],
                            uvb_d.ap()[2 * E:].rearrange("(t p) -> p t", p=P))
        qs_t = singles.tile([P, 1], F32)
        nc.scalar.dma_start(qs_t, qs_d.ap().rearrange("(t p) -> p t", p=P))
        qb_t = singles.tile([P, 1], F32)
        nc.scalar.dma_start(qb_t, qb_d.ap().rearrange("(t p) -> p t", p=P))
        ks_t = singles.tile([P, 1], F32)
        nc.scalar.dma_start(ks_t, ks_d.ap().rearrange("(t p) -> p t", p=P))
        kb_t = singles.tile([P, 1], F32)
        nc.scalar.dma_start(kb_t, kb_d.ap().rearrange("(t p) -> p t", p=P))
        # v bias broadcast (x64 psum domain), on the scalar queue
        uvb_v_bc = singles.tile([P, E], F32)
        _bcast_load(nc, uvb_v_bc, uvbv_d.ap())

        # ---- persistent activations ----
        hT = singles.tile([P, HT, R], F8)     # transposed LN output (fp8)
        uT = singles.tile([P, UT, R], F8)     # u, later y = u*attn in place
        baseT = singles.tile([P, R], F32)
        qT = singles.tile([P, R], F8)
        kT_sb = singles.tile([P, R], F8)
        kT_full = singles.tile([P, KT // RT, R], F8)   # [S, chunk, key]
        g_sb = singles.tile([P, KT, R], F8)            # [key, kt, row]
        wo = singles.tile([P, UT, H], F8)              # o weights, whole

        # ================= Stage 1: LayerNorm (transposed layout) =========
        # x arrives host-transposed as xT [H, R]; stats are computed by
        # contracting the partition (hidden) dim with a ones vector on the
        # PE (fp32 sum chain — the tensor engine is idle this early), then
        # broadcast back over partitions. The normalize is split: subtract
        # on gpsimd (can start as soon as mu lands), multiply on vector.
        ones_f = singles.tile([P, P], F32)
        nc.vector.memset(ones_f, 1.0)
        ones_b = singles.tile([P, P], BF16)
        nc.vector.memset(ones_b, 1.0)
        xT = singles.tile([P, HT, R], F32)
        xsq = singles.tile([P, HT, R], BF16)
        xtr3 = xtr[:].rearrange("(t p) r -> p t r", p=P)
        for hc in range(4):
            nc.sync.dma_start(xT[:, 2 * hc:2 * hc + 2, :],
                              xtr3[:, 2 * hc:2 * hc + 2, :])
        for ht in range(HT):
            nc.vector.tensor_tensor(xsq[:, ht, :], xT[:, ht, :],
                                    xT[:, ht, :], OP.mult)
        psum_s = ps.tile([P, R], F32, tag="mm", name="psum_s")
        psum_q = ps.tile([P, R], F32, tag="mm", name="psum_q")
        for ht in range(HT):
            nc.tensor.matmul(psum_s, ones_f, xT[:, ht, :],
                             start=(ht == 0), stop=(ht == HT - 1))
        for ht in range(HT):
            nc.tensor.matmul(psum_q, ones_b, xsq[:, ht, :],
                             start=(ht == 0), stop=(ht == HT - 1))
        mu_bc = singles.tile([P, R], F32)
        nc.vector.tensor_scalar_mul(mu_bc, psum_s, 1.0 / H)
        rstd_bc = singles.tile([P, R], F32)
        nc.vector.tensor_scalar_mul(rstd_bc, psum_q, 1.0 / H)
        mu2 = singles.tile([P, R], F32)
        nc.vector.tensor_tensor(mu2, mu_bc, mu_bc, OP.mult)
        nc.vector.tensor_tensor(rstd_bc, rstd_bc, mu2, OP.subtract)
        nc.scalar.activation(out=rstd_bc, in_=rstd_bc, func=AF.Sqrt,
                             bias=eps_t, scale=1.0)
        nc.vector.reciprocal(out=rstd_bc, in_=rstd_bc)
        for ht in range(HT):
            nc.vector.tensor_tensor(xT[:, ht, :], xT[:, ht, :], mu_bc,
                                    OP.subtract)
            nc.vector.tensor_tensor(hT[:, ht, :], xT[:, ht, :], rstd_bc,
                                    OP.mult)

        def proj_mm(psum, w_pairs_of, moving_rows=None):
            """4 DR matmuls accumulating h-pair contractions into psum."""
            for hp in range(HP):
                mov = (hT[:, 2 * hp:2 * hp + 2, :] if moving_rows is None
                       else hT[:, 2 * hp:2 * hp + 2, moving_rows])
                nc.tensor.matmul(psum, w_pairs_of(hp), mov, perf_mode=DR,
                                 start=(hp == 0), stop=(hp == HP - 1))

        # ================= Stage 2a: base -> q,k; fire k gather ===========
        wbase = singles.tile([P, HT, S], F8)
        nc.sync.dma_start(wbase,
                          uv_wtr[:, 2 * E:].rearrange("(t p) c -> p t c", p=P))
        pb = ps.tile([P, R], F32, tag="mm")
        proj_mm(pb, lambda hp: wbase[:, 2 * hp:2 * hp + 2, :])
        nc.scalar.activation(out=baseT, in_=pb, func=AF.Silu,
                             bias=uvb_u[:, UT:UT + 1], scale=ISCALE)
        nc.vector.tensor_scalar(out=qT, in0=baseT, scalar1=qs_t, scalar2=qb_t,
                                op0=OP.mult, op1=OP.add)
        nc.vector.tensor_scalar(out=kT_sb, in0=baseT, scalar1=ks_t,
                                scalar2=kb_t, op0=OP.mult, op1=OP.add)
        nc.gpsimd.dma_start(kv_c[:, :R], kT_sb)

        # ================= Stage 2b: v (natural layout), chunked gathers ==
        # v psum is [rows, vcols] (hT pairs stationary, weights moving);
        # each 512-col chunk is stored+gathered as soon as it's done.
        # gpsimd carries ONLY contribution stores + triggers; the
        # gather-dependent reloads ride the idle sync/scalar HWDGE queues.
        for j in range(VCH):
            wv = wpool.tile([P, HT, VC], F8, tag="wuv")
            nc.scalar.dma_start(
                wv, uv_wtr[:, E + j * VC:E + (j + 1) * VC]
                .rearrange("(t p) c -> p t c", p=P))
            v_sb = wpool.tile([P, RT, VC], F8, tag="vsb", name=f"v_sb{j}")
            for rt in range(RT):
                pv = ps.tile([P, VC], F32, tag="mm")
                for hp in range(HP):
                    nc.tensor.matmul(
                        pv, hT[:, 2 * hp:2 * hp + 2, rt * P:(rt + 1) * P],
                        wv[:, 2 * hp:2 * hp + 2, :], perf_mode=DR,
                        start=(hp == 0), stop=(hp == HP - 1))
                vtmp = tmp.tile([P, VC], F32, tag="vtmp")
                nc.vector.tensor_tensor(vtmp, pv,
                                        uvb_v_bc[:, j * VC:(j + 1) * VC],
                                        OP.add)
                nc.scalar.activation(out=v_sb[:, rt, :], in_=vtmp,
                                     func=AF.Silu, scale=ISCALE)
            if j == 0:
                nc.gpsimd.dma_start(
                    kv_c[:, R:].rearrange("p (t c) -> p t c", t=RT), v_sb)
                nc.gpsimd.collective_compute(
                    "AllGather", OP.bypass,
                    replica_groups=[list(range(N_CORES))],
                    ins=[kv_c.opt()], outs=[kv_g.opt()])
            else:
                nc.gpsimd.dma_start(
                    v_c[j][:].rearrange("p (t c) -> p t c", t=RT), v_sb)
                nc.gpsimd.collective_compute(
                    "AllGather", OP.bypass,
                    replica_groups=[list(range(N_CORES))],
                    ins=[v_c[j].opt()], outs=[v_g[j].opt()])

        # ================= Stage 2c: u (fills the gather shadow) ==========
        for ug in range(4):
            wu = wpool.tile([P, HT, 512], F8, tag="wuv")
            nc.sync.dma_start(
                wu, uv_wtr[:, ug * 512:(ug + 1) * 512]
                .rearrange("(t p) c -> p t c", p=P))
            for ui in range(4):
                ut = ug * 4 + ui
                pu = ps.tile([P, R], F32, tag="mm")
                proj_mm(pu, lambda hp: wu[:, 2 * hp:2 * hp + 2,
                                          ui * P:(ui + 1) * P])
                nc.scalar.activation(out=uT[:, ut, :], in_=pu, func=AF.Silu,
                                     bias=uvb_u[:, ut:ut + 1], scale=ISCALE)

        # o weights + residual: loaded behind the u weights on sync
        nc.sync.dma_start(wo, o_wtr[:].rearrange("(t p) c -> p t c", p=P))
        xpb = singles.tile([P, RT, H], F32)
        nc.sync.dma_start(xpb, xpb_d.ap().rearrange("(t p) c -> p t c", p=P))

        # gather reloads: vchunks on sync (idle until the out stores),
        # kT_full on scalar (only gates the score relus, which need the
        # gather anyway). 2KB-contiguous rows on both sides.
        vchunks = []
        for j in range(VCH):
            vchunk = wpool.tile([P, N_CORES, RT, VC], F8, tag="vchunk",
                                name=f"vchunk{j}")
            src = (kv_g[:][:, R:] if j == 0 else v_g[j][:])
            nc.sync.dma_start(
                vchunk[:].rearrange("p c q e -> p c (q e)"),
                src.rearrange("(c p) r -> p c r", p=P))
            vchunks.append(vchunk)
        nc.scalar.dma_start(
            kT_full, kv_g[:][:, :R].rearrange("(c p) r -> p c r", p=P))

        # ================= Stage 3: scores + relu^2 =======================
        # kT_full rows for core c live at kv_g[c*P:(c+1)*P, :R].
        for kt in range(KT):
            c, rb = kt // RT, kt % RT
            pg = ps.tile([P, R], F32, tag="mm")
            nc.tensor.matmul(pg, kT_full[:, c, rb * P:(rb + 1) * P],
                             qT[:], start=True, stop=True)
            t_relu = tmp.tile([P, R], F32, tag="relu", bufs=4)
            nc.scalar.activation(out=t_relu, in_=pg, func=AF.Relu)
            nc.vector.tensor_tensor(g_sb[:, kt, :], t_relu, pg, OP.mult)

        # ================= Stage 4: attn = g @ v; y = u * attn ===========
        # fp8 DoubleRow: stationary = v key-pair stripes, moving = g pairs.
        for j in range(VCH):
            vchunk = vchunks[j]
            pa = [ps.tile([P, R], F32, tag="mm", name=f"pa{j}_{ei}")
                  for ei in range(VC // P)]
            for kp in range(KT // 2):
                c8, rp = kp // 2, kp % 2
                gpair = g_sb[:, 2 * kp:2 * kp + 2, :]
                for ei in range(VC // P):
                    nc.tensor.matmul(
                        pa[ei], vchunk[:, c8, 2 * rp:2 * rp + 2,
                                       ei * P:(ei + 1) * P],
                        gpair, perf_mode=DR,
                        start=(kp == 0), stop=(kp == KT // 2 - 1))
            for ei in range(VC // P):
                et = j * (VC // P) + ei
                nc.vector.tensor_tensor(uT[:, et, :], pa[ei], uT[:, et, :],
                                        OP.mult)

        # ================= Stage 5: out = y @ o_w.T * 2^-22 + (x + o_b) ==
        for hc in range(2):
            for rt in range(RT):
                po = ps.tile([P, 512], F32, tag="mm")
                for ep in range(UT // 2):
                    nc.tensor.matmul(
                        po, uT[:, 2 * ep:2 * ep + 2, rt * P:(rt + 1) * P],
                        wo[:, 2 * ep:2 * ep + 2, hc * 512:(hc + 1) * 512],
                        perf_mode=DR,
                        start=(ep == 0), stop=(ep == UT // 2 - 1))
                o_sb = tmp.tile([P, 512], F32, tag="osb")
                nc.scalar.mul(o_sb, po, OSCALE)
                nc.vector.tensor_tensor(o_sb, o_sb,
                                        xpb[:, rt, hc * 512:(hc + 1) * 512],
                                        OP.add)
                nc.sync.dma_start(
                    outr[rt * P:(rt + 1) * P, hc * 512:(hc + 1) * 512], o_sb)

    nc.finalize()
    return nc


_NC_CACHE = None


def _get_nc():
    global _NC_CACHE
    if _NC_CACHE is None:
        _NC_CACHE = build()
    return _NC_CACHE


def _f8(a):
    import ml_dtypes
    return np.ascontiguousarray(
        np.clip(a, -240.0, 240.0)).astype(ml_dtypes.float8_e4m3fn)


def _make_in_maps(inputs):
    x = np.ascontiguousarray(inputs["x"], dtype=np.float32)
    uv_w = np.asarray(inputs["uv_w"], dtype=np.float32)
    o_w = np.asarray(inputs["o_w"], dtype=np.float32)
    qk_w = np.asarray(inputs["qk_weight"], dtype=np.float32)
    qk_b = np.asarray(inputs["qk_bias"], dtype=np.float32)
    gamma = np.asarray(inputs["ln_gamma"], dtype=np.float32)
    beta = np.asarray(inputs["ln_beta"], dtype=np.float32)
    uv_b = np.asarray(inputs["uv_b"], dtype=np.float32)
    o_b = np.asarray(inputs["o_b"], dtype=np.float32)
    scale = np.float32(1.0 / np.sqrt(np.float32(128.0)))

    # fold gamma/beta into the uv projection:
    #   (z*gamma + beta) @ W.T = z @ (W*gamma).T + W@beta
    uv_w_f = uv_w * gamma[None, :]
    uv_b_f = (uv_b.astype(np.float64)
              + uv_w.astype(np.float64) @ beta.astype(np.float64)
              ).astype(np.float32)

    shared = dict(
        uv_wt=_f8(uv_w_f.T * WLIFT),
        o_wt=_f8(o_w.T * WLIFT),
        uvb=np.ascontiguousarray(uv_b_f),
        uvb_v64=np.ascontiguousarray(uv_b_f[E:2 * E] * WLIFT),
        qs=np.ascontiguousarray(qk_w[0] * scale * QKS),
        qb=np.ascontiguousarray(qk_b[0] * scale * QKS),
        ks=np.ascontiguousarray(qk_w[1] * QKS),
        kb=np.ascontiguousarray(qk_b[1] * QKS),
    )
    return [dict(shared,
                 xt=np.ascontiguousarray(x[c * R:(c + 1) * R].T),
                 xpb=np.ascontiguousarray(x[c * R:(c + 1) * R] + o_b))
            for c in range(N_CORES)]


def run(inputs, trace=False, **kw):
    nc = _get_nc()
    in_maps = _make_in_maps(inputs)
    res = run_bass_kernel_spmd(nc, in_maps, list(range(N_CORES)),
                               trace=trace, **kw)
    out = np.concatenate([res.results[c]["out"] for c in range(N_CORES)],
                         axis=0)
    return out, res


def kernel(**inputs) -> np.ndarray:
    out, _ = run(inputs)
    return out
